# revision 13
# baseline (speedup 1.0000x reference)
"""Trainium2 Bass kernel for nn_Net_91268055040039 (dense_mlp).

Computes out[b] = sum_{t,p} x[b,t,p] * |W[t,p]| * fc1_w[0, t*P+p] + fc1_b
  x: [32, 400, 10000] f32, W: [400, 10000] f32, fc1_w: [1, 4000000] f32.

Strategy: shard the reduction dim T=400 into 8 slices of 50 rows. The whole
problem is HBM-bound (x alone is 512 MB f32; the 8 cores of one trn2 chip
share ~2.9 TB/s), so inputs are uploaded in fp16 (max rel err 5e-3 vs the
f64 oracle; tolerance 2e-2): 32 MB of x + 2 MB of params per core -> ~92 us
DMA floor at the ~358-375 GB/s per-core HBM share.

Measured engine rates for the multiply+reduce of one batch (3908 elems/lane):
  DVE fused scalar_tensor_tensor+accum: 4.55 us (1x only)
  DVE tensor_tensor mult (fp16 2x_1P):  2.05 us/batch done in pairs
  ACT activation(Identity, accum_out):  3.72 us (accum forces 1x)
Split 24 batches across DVE-mult+ACT-reduce and 8 batches fused on DVE so
both engines finish with the stream. x chunks alternate between the sync
(HWDGE) and gpsimd (SWDGE) DMA rings; first chunks are small (2 batches) to
cut the ramp, last chunks small to cut the tail.

Host sums the 8 per-core partials and adds fc1_b.
"""

import numpy as np

import concourse.bass as bass
import concourse.bacc as bacc
import concourse.mybir as mybir
from concourse.tile import TileContext
from concourse.bass_utils import run_bass_kernel_spmd

B, T, P = 32, 400, 10000
NCORES = 8
TS = T // NCORES          # 50 T-rows per core
K = TS * P                # 500000 reduction elements per core per batch
PART = 128
FREE = 3908               # even so batch slices stay 4B-aligned (padded K/128)
KPAD = PART * FREE        # 500224 (224 zero pad)
F32 = mybir.dt.float32
F16 = mybir.dt.float16

# chunk schedule: (n_batches, mode) where mode 's'=split pairs (DVE TT +
# ACT reduce), 'f'=fused STT on DVE, 'sf'=1 split pair + 2 fused,
# 'g'=split pair with the multiply on gpsimd (probe).
SCHEDULE = [
    (2, "s"), (2, "s"),
    (4, "ss"), (4, "ss"), (4, "ss"), (4, "ss"),
    (4, "sf"), (4, "sf"),
    (2, "f"), (1, "f"), (1, "f"),
]
assert sum(n for n, _ in SCHEDULE) == B

PE_PROBE_GROUPS = 0

# Set by the test harness to capture an NTFF profile; harmless when False.
TRACE = False
LAST_RESULT = None


def build_program() -> bass.Bass:
    # Bacc (not raw Bass): its compile() splits multi-sem waits into separate
    # instructions - this neuronxcc build allows only 1 sync-wait per inst.
    nc = bacc.Bacc()
    xs = nc.declare_dram_parameter("xs", [PART, B * FREE], F16, isOutput=False)
    # wf[:, :FREE] = W shard, wf[:, FREE:] = fc1 shard (one DMA for both).
    wf = nc.declare_dram_parameter("wf", [PART, 2 * FREE], F16, isOutput=False)
    out = nc.declare_dram_parameter("out", [1, B], F32, isOutput=True)

    with TileContext(nc) as tc:
        with (
            tc.tile_pool(name="const", bufs=1) as cpool,
            tc.tile_pool(name="x4", bufs=2) as x4pool,
            tc.tile_pool(name="x2", bufs=2) as x2pool,
            tc.tile_pool(name="x1", bufs=2) as x1pool,
            tc.tile_pool(name="sc", bufs=2) as spool,
            tc.tile_pool(name="psum", bufs=1, space="PSUM") as ppool,
            tc.tile_pool(name="psprobe", bufs=1, space="PSUM") as probepool,
        ):
            # Params FIRST on the sync ring, ahead of the x chunks on that
            # ring (HWDGE FIFO): a small transfer racing the big x packets on
            # another ring gets starved by packet round-robin (~20 us).
            wft = cpool.tile([PART, 2 * FREE], F16)
            nc.sync.dma_start(out=wft, in_=wf[:, :])
            # v = |W| * fc1 in place over the W half, then duplicated to
            # [v | v] so one TT op can cover two batches.
            v = wft[:, :FREE]
            nc.scalar.activation(
                out=v, in_=v, func=mybir.ActivationFunctionType.Abs
            )
            nc.vector.tensor_tensor(
                out=v, in0=v, in1=wft[:, FREE:], op=mybir.AluOpType.mult
            )
            v2 = cpool.tile([PART, 2 * FREE], F16)
            nc.vector.tensor_copy(out=v2[:, :FREE], in_=v)
            nc.vector.tensor_copy(out=v2[:, FREE:], in_=v)

            ones = cpool.tile([PART, 1], F32)
            nc.vector.memset(ones, 1.0)
            acc = cpool.tile([PART, B], F32)

            def split_pair(xt, c, b, engine):
                """multiply batches (b, b+1) on `engine`, reduce on ACT."""
                prod = spool.tile([PART, 2 * FREE], F16, tag="prod")
                engine.tensor_tensor(
                    out=prod,
                    in0=xt[:, c * FREE : (c + 2) * FREE],
                    in1=v2,
                    op=mybir.AluOpType.mult,
                )
                for i in range(2):
                    half = prod[:, i * FREE : (i + 1) * FREE]
                    nc.scalar.activation(
                        out=half,
                        in_=half,
                        func=mybir.ActivationFunctionType.Identity,
                        accum_out=acc[:, b + i : b + i + 1],
                    )

            def fused(xt, c, b):
                """multiply+reduce batch b in one DVE STT (1x mode)."""
                sl = xt[:, c * FREE : (c + 1) * FREE]
                nc.vector.scalar_tensor_tensor(
                    out=sl,
                    in0=sl,
                    scalar=0.0,
                    in1=v,
                    op0=mybir.AluOpType.bypass,
                    op1=mybir.AluOpType.mult,
                    accum_out=acc[:, b : b + 1],
                )

            pools = {4: x4pool, 2: x2pool, 1: x1pool}
            rings = [nc.sync, nc.gpsimd]
            b = 0
            for ci, (n, mode) in enumerate(SCHEDULE):
                xt = pools[n].tile([PART, n * FREE], F16, tag=f"x{n}")
                rings[ci % 2].dma_start(
                    out=xt, in_=xs[:, b * FREE : (b + n) * FREE]
                )
                c = 0
                for m in mode:
                    if m == "s":
                        split_pair(xt, c, b + c, nc.vector)
                        c += 2
                    elif m == "g":
                        split_pair(xt, c, b + c, nc.gpsimd)
                        c += 2
                    elif m == "f":
                        for _ in range(min(2, n - c) or 1):
                            if c < n:
                                fused(xt, c, b + c)
                                c += 1
                assert c == n, (ci, mode, c, n)
                b += n

            if PE_PROBE_GROUPS:
                pprobe = probepool.tile([PART, 512], F32)
                for g in range(PE_PROBE_GROUPS):
                    for j in range(8):
                        nc.tensor.matmul(
                            out=pprobe,
                            lhsT=v2[:, g * 128 : (g + 1) * 128],
                            rhs=v2[:, j * 512 : (j + 1) * 512],
                            start=(g == 0 and j == 0),
                            stop=(g == PE_PROBE_GROUPS - 1 and j == 7),
                        )

            ps = ppool.tile([1, B], F32)
            nc.tensor.matmul(out=ps, lhsT=ones, rhs=acc, start=True, stop=True)
            res = cpool.tile([1, B], F32)
            nc.scalar.copy(res, ps)
            nc.sync.dma_start(out=out[:, :], in_=res)
    nc.finalize()
    return nc


def _to_partition_major_f16(flat: np.ndarray) -> np.ndarray:
    """[N, K] row-major -> fp16 [PART, N*FREE] where each partition's rows for
    consecutive N are adjacent (N along the middle axis)."""
    n = flat.shape[0]
    padded = np.zeros((n, KPAD), dtype=np.float16)
    padded[:, :K] = flat  # f32 -> fp16 cast happens here
    # [n, PART, FREE] -> [PART, n, FREE] -> [PART, n*FREE]
    return np.ascontiguousarray(
        padded.reshape(n, PART, FREE).transpose(1, 0, 2)
    ).reshape(PART, n * FREE)


def make_in_maps(x: np.ndarray, W: np.ndarray, fc1_w: np.ndarray):
    x = np.asarray(x, dtype=np.float32)
    W = np.asarray(W, dtype=np.float32)
    fc1_w = np.asarray(fc1_w, dtype=np.float32)
    fc1_flat = fc1_w.reshape(T, P)
    in_maps = []
    for c in range(NCORES):
        t0 = c * TS
        xs = _to_partition_major_f16(x[:, t0 : t0 + TS, :].reshape(B, K))
        ws = _to_partition_major_f16(W[t0 : t0 + TS, :].reshape(1, K))
        fs = _to_partition_major_f16(fc1_flat[t0 : t0 + TS, :].reshape(1, K))
        in_maps.append({"xs": xs, "wf": np.concatenate([ws, fs], axis=1)})
    return in_maps


def kernel(x, W, fc1_w, fc1_b):
    global LAST_RESULT
    nc = build_program()
    in_maps = make_in_maps(x, W, fc1_w)
    res = run_bass_kernel_spmd(
        nc, in_maps, core_ids=list(range(NCORES)), trace=TRACE
    )
    LAST_RESULT = res
    partial = np.zeros(B, dtype=np.float64)
    for r in res.results:
        partial += r["out"][0].astype(np.float64)
    out = partial.astype(np.float32) + np.float32(np.asarray(fc1_b).reshape(-1)[0])
    return out.reshape(B, 1).astype(np.float32)
